# revision 14
# baseline (speedup 1.0000x reference)
"""Trainium2 Bass kernel for nn_PixelRNN: 16x16-step LSTM recurrence over a
4096 batch, data-parallel across 8 NeuronCores (512 batch each).

Self-contained: host-side weight folding + Bass/Tile module + SPMD run.

On-chip math (per step c in 8..23 of each row, all fp16 in SBUF, fp32 PSUM):
  pre   = lhsT_S(var).T @ S + lhsT_R(var).T @ R      (2 PSUM halves A=[i;g], B=[f;o])
  T     = tanh(pre)            # weights pre-scaled: i,f,o rows 0.5x, g rows 1x
  [u|w] = (T_top + 1) * [Tg|CC]                      # one fused STT
  CC'   = 0.5*w + u            # CC == 2*cx
  ts    = tanh(0.5*CC')
  H'    = (To + 1) * ts        # H == 2*hx  (H-columns of weights pre-scaled 0.5x)
  y     = lhsT_v.T @ S ; v = leaky(y) -> feed slot (+ DRAM out)
The 8-value feed window lives in: a 24-partition rotation-indexed region of S
(ages 6-8), plus 5 "slots" at 32-aligned partition bases (ages 1-5), with a
small DMA consolidating each v into the rotation region 6 steps later.
16 precomputed weight variants (one per in-row step) encode the slot/rotation
mapping, zeroing inactive rows.
"""

import numpy as np

RS, L, R, HID = 8, 8, 24, 64
SLOPE = 0.01
W_IMG = 24
NROWS = 16  # rows L..R-1
NCOLS = 16  # cols L..R-1
NVAR = 16

# ---------------------------------------------------------------------------
# host-side weight folding
# ---------------------------------------------------------------------------


def _orig_rows():
    """out-row index (half, m) -> original gate row in [0, 4*HID).
    half A = [i; f], half B = [g; o] (so every 2-input SBUF op has
    operand base partitions equal, per the walrus verifier rule)."""
    a = np.arange(0, 2 * HID)
    b = np.arange(2 * HID, 4 * HID)
    return a, b


def _rowscale():
    sa = np.full(2 * HID, 0.5)  # i, f
    sb = np.concatenate([np.full(HID, 1.0), np.full(HID, 0.5)])  # g, o
    return sa, sb


def _slot_info(c):
    """For step c: per slot s in 0..4 the (value col c_s, age a_s)."""
    out = []
    for s in range(5):
        cs = (c - 1) - ((c - 1 - s) % 5)
        out.append((cs, c - cs))
    return out


def prep_weights(W_ih, W_hh, b_ih, b_hh, Wl, bl):
    """Returns WV [128, 2*NVAR*2*128 + 3] float16:
    cols (v*2+h)*128           : lhsT_S variant v half h
    cols 4096 + (v*2+h)*128    : lhsT_R variant v half h
    cols 8192:8195             : lhsT_v
    """
    oa, ob = _orig_rows()
    sa, sb = _rowscale()
    origs = [oa, ob]
    scales = [sa, sb]
    bias = (b_ih + b_hh).astype(np.float64)
    Wih = W_ih.astype(np.float64)
    Whh = W_hh.astype(np.float64)

    WV = np.zeros((128, 2 * NVAR * 2 * 128 + 3), np.float64)

    for v in range(NVAR):
        c = 8 + v
        slots = _slot_info(c)
        for h in range(2):
            om, sc = origs[h], scales[h]
            lS = np.zeros((128, 128))
            lR = np.zeros((128, 128))
            # H-part (stored H = 2*hx -> extra 0.5)
            lS[0:64, :] = 0.5 * (Whh[om, :] * sc[:, None]).T
            # rotation window region (q-major: 64 + 3q + ch), ages 6..8
            for a in (6, 7, 8):
                j = 8 - a
                q = (c - a) % 8
                for ch in range(3):
                    lS[64 + 3 * q + ch, :] = sc * Wih[om, ch * 8 + j]
            # bias row
            lS[88, :] = sc * bias[om]
            # slot 4 (in S at partitions 96:99)
            c4, a4 = slots[4]
            j4 = 8 - a4
            for ch in range(3):
                lS[96 + ch, :] = sc * Wih[om, ch * 8 + j4]
            # slots 0..3 in R at partition bases 32*s
            for s in range(4):
                cs, a = slots[s]
                j = 8 - a
                for ch in range(3):
                    lR[32 * s + ch, :] = sc * Wih[om, ch * 8 + j]
            WV[:, (v * 2 + h) * 128 : (v * 2 + h + 1) * 128] = lS
            WV[:, 4096 + (v * 2 + h) * 128 : 4096 + (v * 2 + h + 1) * 128] = lR

    lv = np.zeros((128, 3))
    lv[0:64, :] = 0.5 * Wl.astype(np.float64).T
    lv[88, :] = bl.astype(np.float64)
    WV[:, 8192:8195] = lv
    return WV.astype(np.float16)


def prep_x(x_core):
    """x_core [Bc, 3, 24, 24] fp32 -> xT [8, 3, NROWS, Bc] fp16 (cols 0..7 of
    rows L..R-1, col-major so the q-major window region loads contiguously)."""
    xs = x_core[:, :, L:R, 0:RS]  # [Bc, 3, 16, 8]
    return np.ascontiguousarray(xs.transpose(3, 1, 2, 0)).astype(np.float16)


# ---------------------------------------------------------------------------
# bass module
# ---------------------------------------------------------------------------


def build_module(Bc=512, G=2, nrows=NROWS, dt_f=None, leaky_lrelu=False):
    import concourse.bacc as bacc
    import concourse.tile as tile
    from concourse import mybir

    fp = dt_f or mybir.dt.float16
    f32 = mybir.dt.float32
    Bg = Bc // G
    AF = mybir.ActivationFunctionType
    OP = mybir.AluOpType

    nc = bacc.Bacc("TRN2", target_bir_lowering=False, debug=False, num_devices=8)

    xT = nc.dram_tensor("xT", [RS, 3, nrows, Bc], fp, kind="ExternalInput")
    WV = nc.dram_tensor("WV", [128, 2 * NVAR * 2 * 128 + 3], fp, kind="ExternalInput")
    ones = nc.dram_tensor("ones", [1, Bc], fp, kind="ExternalInput")
    outT = nc.dram_tensor("outT", [nrows, NCOLS, 3, Bc], fp, kind="ExternalOutput")

    with tile.TileContext(nc) as tc:
        with (
            tc.tile_pool(name="wpool", bufs=1) as wpool,
            tc.tile_pool(name="state", bufs=1) as state,
            tc.tile_pool(name="psum", bufs=1, space="PSUM") as pp,
        ):
            wv = wpool.tile([128, 2 * NVAR * 2 * 128 + 3], fp, name="wv", tag="wv")
            nc.sync.dma_start(out=wv, in_=WV[:, :])

            def colS(v, h):
                return wv[:, (v * 2 + h) * 128 : (v * 2 + h + 1) * 128]

            def colR(v, h):
                return wv[:, 4096 + (v * 2 + h) * 128 : 4096 + (v * 2 + h + 1) * 128]

            wv_v = wv[:, 8192:8195]

            # per-group persistent tiles
            Ss, Rs, Ts, Ps, TSs, PSg, PSv = [], [], [], [], [], [], []
            for g in range(G):
                Ss.append(
                    [state.tile([128, Bg], fp, name=f"S{g}{p}", tag=f"S{g}{p}") for p in range(2)]
                )
                Rs.append([state.tile([128, Bg], fp, name=f"R{g}{p}", tag=f"R{g}{p}") for p in range(2)])
                Ts.append(state.tile([128, 3 * Bg], fp, name=f"T{g}", tag=f"T{g}"))
                Ps.append(state.tile([64, 2 * Bg], fp, name=f"P{g}", tag=f"P{g}"))
                TSs.append(state.tile([128, Bg], fp, name=f"TS{g}", tag=f"TS{g}"))
                PSg.append(pp.tile([128, 2 * Bg], f32, name=f"PSg{g}", tag=f"PSg{g}"))
                PSv.append(pp.tile([3, Bg], f32, name=f"PSv{g}", tag=f"PSv{g}"))

            gsl = lambda g: slice(g * Bg, (g + 1) * Bg)

            # init: zero state tiles (H=0, CC=0, unused rows), ones-row
            for g in range(G):
                for p in range(2):
                    nc.vector.memset(Ss[g][p][:, :], 0.0)
                    nc.vector.memset(Rs[g][p][:, :], 0.0)
                    nc.sync.dma_start(out=Ss[g][p][88:89, :], in_=ones[:, gsl(g)])
                nc.vector.memset(Ts[g][64:128, 2 * Bg : 3 * Bg], 0.0)

            def preload_row(ridx):
                p = ridx % 2
                for g in range(G):
                    nc.sync.dma_start(
                        out=Ss[g][p][64:88, :],
                        in_=xT[:, :, ridx, gsl(g)].rearrange("j c b -> (j c) b"),
                    )
                    for s, col in ((0, 5), (1, 6), (2, 7), (3, 3)):
                        nc.sync.dma_start(
                            out=Rs[g][p][32 * s : 32 * s + 3, :],
                            in_=xT[col, :, ridx, gsl(g)],
                        )
                    nc.sync.dma_start(
                        out=Ss[g][p][96:99, :], in_=xT[4, :, ridx, gsl(g)]
                    )

            preload_row(0)
            if nrows > 1:
                preload_row(1)

            for ridx in range(nrows):
                p = ridx % 2
                for ci in range(NCOLS):
                    c = 8 + ci
                    last = ci == NCOLS - 1
                    ctx = []
                    for g in range(G):
                        S, Rt = Ss[g][p], Rs[g][p]
                        Snext = Ss[g][(ridx + 1) % 2] if last else S
                        s = c % 5
                        slot = (
                            S[96:99, :] if s == 4 else Rt[32 * s : 32 * s + 3, :]
                        )
                        ctx.append((S, Rt, Ts[g], Ps[g], TSs[g], PSg[g],
                                    PSv[g], Snext, slot))

                    # phase-interleaved emission across groups so each
                    # engine's in-order queue never head-of-line blocks on
                    # the other group's not-yet-ready op
                    for g, (S, Rt, T, P, TS, PS, PV, Snext, slot) in enumerate(ctx):
                        # gate matmuls: A=[i;g] cols 0:Bg, B=[f;o] cols Bg:2Bg
                        for h in range(2):
                            o = PS[:, h * Bg : (h + 1) * Bg]
                            nc.tensor.matmul(
                                out=o, lhsT=colS(ci, h), rhs=S[:, :],
                                start=True, stop=False,
                            )
                            nc.tensor.matmul(
                                out=o, lhsT=colR(ci, h), rhs=Rt[:, :],
                                start=False, stop=True,
                            )
                    for g, (S, Rt, T, P, TS, PS, PV, Snext, slot) in enumerate(ctx):
                        # T = tanh(pre) over both halves in one op
                        nc.scalar.activation(
                            out=T[:, 0 : 2 * Bg], in_=PS[:, :], func=AF.Tanh
                        )
                    for g, (S, Rt, T, P, TS, PS, PV, Snext, slot) in enumerate(ctx):
                        # u = (Ti + 1) * Tg   (both operands base partition 0)
                        nc.vector.scalar_tensor_tensor(
                            out=P[:, 0:Bg],
                            in0=T[0:64, 0:Bg],
                            scalar=1.0,
                            in1=T[0:64, Bg : 2 * Bg],
                            op0=OP.add,
                            op1=OP.mult,
                        )
                    for g, (S, Rt, T, P, TS, PS, PV, Snext, slot) in enumerate(ctx):
                        # w = (Tf + 1) * CC   (both operands base partition 64)
                        nc.vector.scalar_tensor_tensor(
                            out=P[:, Bg : 2 * Bg],
                            in0=T[64:128, 0:Bg],
                            scalar=1.0,
                            in1=T[64:128, 2 * Bg : 3 * Bg],
                            op0=OP.add,
                            op1=OP.mult,
                        )
                    for g, (S, Rt, T, P, TS, PS, PV, Snext, slot) in enumerate(ctx):
                        # CC' = 0.5*w + u
                        nc.vector.scalar_tensor_tensor(
                            out=T[64:128, 2 * Bg : 3 * Bg],
                            in0=P[:, Bg : 2 * Bg],
                            scalar=0.5,
                            in1=P[:, 0:Bg],
                            op0=OP.mult,
                            op1=OP.add,
                        )
                    for g, (S, Rt, T, P, TS, PS, PV, Snext, slot) in enumerate(ctx):
                        # ts = tanh(0.5*CC')
                        nc.scalar.activation(
                            out=TS[64:128, :],
                            in_=T[64:128, 2 * Bg : 3 * Bg],
                            func=AF.Tanh,
                            scale=0.5,
                        )
                    for g, (S, Rt, T, P, TS, PS, PV, Snext, slot) in enumerate(ctx):
                        # H' = (To + 1) * ts
                        nc.vector.scalar_tensor_tensor(
                            out=Snext[0:64, :],
                            in0=T[64:128, Bg : 2 * Bg],
                            scalar=1.0,
                            in1=TS[64:128, :],
                            op0=OP.add,
                            op1=OP.mult,
                        )
                    for g, (S, Rt, T, P, TS, PS, PV, Snext, slot) in enumerate(ctx):
                        # v matmul (reads the tile H' was written to)
                        nc.tensor.matmul(
                            out=PV[:, :], lhsT=wv_v, rhs=Snext[:, :],
                            start=True, stop=True,
                        )
                    for g, (S, Rt, T, P, TS, PS, PV, Snext, slot) in enumerate(ctx):
                        if leaky_lrelu:
                            nc.scalar.activation(
                                out=slot, in_=PV[:, :], func=AF.Lrelu, alpha=SLOPE
                            )
                        else:
                            ysb = TS[0:3, :]
                            nc.scalar.activation(
                                out=ysb, in_=PV[:, :], func=AF.Copy
                            )
                            nc.vector.scalar_tensor_tensor(
                                out=slot, in0=ysb, scalar=SLOPE, in1=ysb,
                                op0=OP.mult, op1=OP.max,
                            )
                    for g, (S, Rt, T, P, TS, PS, PV, Snext, slot) in enumerate(ctx):
                        # DRAM out
                        nc.gpsimd.dma_start(
                            out=outT[ridx, ci, :, gsl(g)], in_=slot
                        )
                        # consolidation into the rotation region (ages 6+)
                        if ci <= 9:
                            q = c % 8
                            nc.gpsimd.dma_start(
                                out=S[64 + 3 * q : 67 + 3 * q, :], in_=slot
                            )
                # prefetch the next-next row (same-parity tiles are free
                # once this row's last reads are done; Tile orders the WAR)
                if ridx + 2 < nrows:
                    preload_row(ridx + 2)

    nc.compile()
    return nc


# ---------------------------------------------------------------------------
# public entry point
# ---------------------------------------------------------------------------

_CACHED = {}


def kernel(x, W_ih, W_hh, b_ih, b_hh, Wl, bl):
    from concourse import bass_utils

    B = x.shape[0]
    NCORES = 8
    Bc = B // NCORES
    x = np.asarray(x, np.float32)

    WV = prep_weights(
        np.asarray(W_ih, np.float32), np.asarray(W_hh, np.float32),
        np.asarray(b_ih, np.float32), np.asarray(b_hh, np.float32),
        np.asarray(Wl, np.float32), np.asarray(bl, np.float32),
    )
    ones = np.ones((1, Bc), np.float16)

    key = (Bc,)
    if key not in _CACHED:
        _CACHED[key] = build_module(Bc=Bc, G=2)
    nc = _CACHED[key]

    in_maps = []
    for cid in range(NCORES):
        xs = x[cid * Bc : (cid + 1) * Bc]
        in_maps.append({"xT": prep_x(xs), "WV": WV, "ones": ones})

    res = bass_utils.run_bass_kernel_spmd(nc, in_maps, core_ids=list(range(NCORES)))
    outs = []
    for cid in range(NCORES):
        ot = res.results[cid]["outT"].astype(np.float32)  # [16,16,3,Bc]
        outs.append(np.ascontiguousarray(ot.transpose(3, 2, 0, 1)))
    return np.concatenate(outs, axis=0)
